# revision 10
# baseline (speedup 1.0000x reference)
"""Multi-head causal attention (B=2, N=2048, D=1024, H=16) on 8 TRN2 NeuronCores.

Sharding: data-parallel over batch (2) x tensor-parallel over head groups (4),
so each core handles one batch element and 4 heads (256 of the 1024 hidden
channels). Wq/Wk/Wv are column-sharded, Wo row-sharded; each core emits a
partial output [2048, 1024] that the host sums over the 4 head groups.

Per-core dataflow (all matmuls bf16 with fp32 PSUM accumulation):
  xT (pre-transposed on host)      [1024, 2048]
  Q^T = Wq_c^T x^T, K^T likewise   [256, 2048]   (head h at partition 64*(h%2), m-tile h//2)
  V   = x Wv_c                     [2048, 256]   stored per seq-tile with an
                                   appended ones column per head (the ones row
                                   of U = exp(S)^T-matmul gives the softmax
                                   denominator without a separate pass)
  S^T tiles = K^T_tile^T Q^T       [128k, 512q]  two heads packed in the PE
                                   array via base partitions 0/64 (K=64 each)
  expS = exp(S^T/8), causal mask applied by multiplying precomputed 0/1 tiles
  U = V_aug^T expS accumulated over k-tiles; row 64 (even head) / 63 (odd) is
      the softmax denominator; ctx^T = U[data] * bcast(1/r)
  Y = ctx^T^T Wo_c                 [2048, 1024] fp32 partial out
No max-subtraction is needed: scores have |S/8| < ~4 for this problem scale.
"""

import sys

sys.path.insert(0, "/opt/trn_rl_repo")

import numpy as np
import ml_dtypes

import concourse.bass as bass
import concourse.bacc as bacc
import concourse.mybir as mybir
from concourse.tile import TileContext
from concourse.bass_utils import run_bass_kernel_spmd

BF16 = mybir.dt.bfloat16
F32 = mybir.dt.float32

B, N, D, H = 2, 2048, 1024, 16
HD = 64          # head dim
HPC = 4          # heads per core
DH = HPC * HD    # 256 hidden channels per core
NCORES = 8
KT = D // 128    # 8 contraction tiles over D
ST = N // 128    # 16 seq tiles
QC = N // 512    # 4 q-chunks of 512

# v_sb per-seq-tile column layout: for each head pair, an "even" block
# [V(64) | ones(1)] (matmul M=65 -> U partitions 0..64, denom at 64) and an
# "odd" block [ones(1) | zeros(63) | V(64)] (M=128 -> U partitions 64..127
# hold data, denom at partition 0, zeros keep partitions 1..63 inert).
V_BLK = {0: (0, 65), 1: (65, 193), 2: (193, 258), 3: (258, 386)}
V_COLS = 386
V_DATA_OFF = {0: 0, 1: 129, 2: 193, 3: 322}

def _build_nc(debug: bool = False) -> bass.Bass:
    nc = bacc.Bacc()
    xT = nc.declare_dram_parameter("xT", [D, N], BF16, isOutput=False)
    wq = nc.declare_dram_parameter("wq", [D, DH], BF16, isOutput=False)
    wk = nc.declare_dram_parameter("wk", [D, DH], BF16, isOutput=False)
    wv = nc.declare_dram_parameter("wv", [D, DH], BF16, isOutput=False)
    wo = nc.declare_dram_parameter("wo", [DH, D], BF16, isOutput=False)
    y = nc.declare_dram_parameter("y", [N, D], F32, isOutput=True)
    if debug:
        dbg = {
            "d_qT": nc.declare_dram_parameter("d_qT", [128, 2 * N], F32, isOutput=True),
            "d_kT": nc.declare_dram_parameter("d_kT", [128, 2 * N], F32, isOutput=True),
            "d_v": nc.declare_dram_parameter("d_v", [128, ST * V_COLS], F32, isOutput=True),
            "d_ctxT": nc.declare_dram_parameter("d_ctxT", [128, 2 * N], F32, isOutput=True),
            "d_u": nc.declare_dram_parameter("d_u", [128, 512], F32, isOutput=True),
            "d_ex": nc.declare_dram_parameter("d_ex", [128, 512], F32, isOutput=True),
            "d_rb": nc.declare_dram_parameter("d_rb", [128, 512], F32, isOutput=True),
        }

    xT_r = xT.rearrange("(t p) n -> t p n", p=128)
    wq_r = wq.rearrange("(t p) m -> t p m", p=128)
    wk_r = wk.rearrange("(t p) m -> t p m", p=128)
    wv_r = wv.rearrange("(t p) m -> t p m", p=128)
    wo_r = wo.rearrange("(t p) m -> t p m", p=128)
    y_r = y.rearrange("(t p) m -> t p m", p=128)

    with TileContext(nc) as tc:
        with (
            tc.tile_pool(name="const", bufs=1) as cpool,
            tc.tile_pool(name="io", bufs=3) as io_pool,
            tc.tile_pool(name="exps", bufs=6) as exp_pool,
            tc.tile_pool(name="small", bufs=4) as small_pool,
            tc.tile_pool(name="ps_mm", bufs=3, space="PSUM") as ps_mm,
            tc.tile_pool(name="ps_u", bufs=2, space="PSUM") as ps_u_pool,
            tc.tile_pool(name="drams", bufs=1, space="DRAM") as dram_pool,
        ):
            rscr = dram_pool.tile([16, 512], F32)
            xT_sb = cpool.tile([128, KT, N], BF16)
            wq_sb = cpool.tile([128, KT, DH], BF16)
            wk_sb = cpool.tile([128, KT, DH], BF16)
            wv_sb = cpool.tile([128, KT, DH], BF16)
            wo_sb = cpool.tile([128, 2, D], BF16)
            qT_sb = cpool.tile([128, 2, N], BF16)
            kT_sb = cpool.tile([128, 2, N], BF16)
            v_sb = cpool.tile([128, ST, V_COLS], BF16)
            ctxT_sb = cpool.tile([128, 2, N], BF16)
            masks_sb = cpool.tile([128, 4, 512], BF16)

            for t in range(KT):
                nc.sync.dma_start(out=xT_sb[:, t, :], in_=xT_r[t])
                nc.sync.dma_start(out=wq_sb[:, t, :], in_=wq_r[t])
                nc.sync.dma_start(out=wk_sb[:, t, :], in_=wk_r[t])
                nc.sync.dma_start(out=wv_sb[:, t, :], in_=wv_r[t])
            for t in range(2):
                nc.sync.dma_start(out=wo_sb[:, t, :], in_=wo_r[t])

            # Causal masks for the 4 diagonal-crossing k-tiles of a q-chunk:
            # keep (1.0) where dq >= dk + 128*i.
            for i in range(4):
                nc.vector.memset(masks_sb[:, i, :], 1.0)
                nc.gpsimd.affine_select(
                    out=masks_sb[:, i, :],
                    in_=masks_sb[:, i, :],
                    compare_op=mybir.AluOpType.is_ge,
                    fill=0.0,
                    base=-(128 * i),
                    pattern=[[1, 512]],
                    channel_multiplier=-1,
                )

            # ones / zeros scaffolding of the V blocks (all seq tiles at once)
            nc.vector.memset(v_sb[:, :, 66:129], 0.0)
            nc.vector.memset(v_sb[:, :, 259:322], 0.0)
            for col in (64, 65, 257, 258):
                nc.vector.memset(v_sb[:, :, col : col + 1], 1.0)

            # ---- Q^T / K^T projections: [256, 2048] each ----
            for w_sb, dst in ((wq_sb, qT_sb), (wk_sb, kT_sb)):
                for mt in range(2):
                    for qc in range(QC):
                        ps = ps_mm.tile([128, 512], F32, tag="mm")
                        for kt in range(KT):
                            nc.tensor.matmul(
                                ps,
                                lhsT=w_sb[:, kt, 128 * mt : 128 * (mt + 1)],
                                rhs=xT_sb[:, kt, 512 * qc : 512 * (qc + 1)],
                                start=(kt == 0),
                                stop=(kt == KT - 1),
                            )
                        nc.vector.tensor_copy(
                            dst[:, mt, 512 * qc : 512 * (qc + 1)], ps
                        )

            # ---- V = x @ Wv_c, stored per seq tile with ones columns ----
            for st in range(ST):
                ps = ps_mm.tile([128, 512], F32, tag="mm")
                psv = ps[:, 0:DH]
                for kt in range(KT):
                    nc.tensor.matmul(
                        psv,
                        lhsT=xT_sb[:, kt, 128 * st : 128 * (st + 1)],
                        rhs=wv_sb[:, kt, :],
                        start=(kt == 0),
                        stop=(kt == KT - 1),
                    )
                ps_h = ps.rearrange("p (h d) -> p h d", d=HD)
                # even heads 0,2 -> offsets 0,193; odd heads 1,3 -> 129,322
                ev = bass.AP(
                    tensor=v_sb.tensor,
                    offset=v_sb[:, st, 0:1].offset,
                    ap=[v_sb.ap[0], [193, 2], [1, HD]],
                )
                od = bass.AP(
                    tensor=v_sb.tensor,
                    offset=v_sb[:, st, 129:130].offset,
                    ap=[v_sb.ap[0], [193, 2], [1, HD]],
                )
                in_ev = bass.AP(
                    tensor=ps.tensor,
                    offset=ps_h[:, 0, :].offset,
                    ap=[ps.ap[0], [2 * HD, 2], [1, HD]],
                )
                in_od = bass.AP(
                    tensor=ps.tensor,
                    offset=ps_h[:, 1, :].offset,
                    ap=[ps.ap[0], [2 * HD, 2], [1, HD]],
                )
                nc.vector.tensor_copy(ev, in_ev)
                nc.vector.tensor_copy(od, in_od)

            # ---- attention: head pair mt (heads 2mt @parts 0-63, 2mt+1 @64-127) ----
            for mt in range(2):
                for qc in range(QC):
                    nkt = 4 * (qc + 1)
                    ps_u = {
                        0: ps_u_pool.tile([128, 512], F32, tag="ue", name="ue"),
                        1: ps_u_pool.tile([128, 512], F32, tag="uo", name="uo"),
                    }
                    for kt in range(nkt):
                        for parity in (0, 1):
                            head = 2 * mt + parity
                            pofs = 64 * parity
                            ps_s = ps_mm.tile([128, 512], F32, tag="mm")
                            nc.tensor.matmul(
                                ps_s,
                                lhsT=kT_sb[
                                    pofs : pofs + 64, mt, 128 * kt : 128 * (kt + 1)
                                ],
                                rhs=qT_sb[
                                    pofs : pofs + 64, mt, 512 * qc : 512 * (qc + 1)
                                ],
                                start=True,
                                stop=True,
                            )
                            ex = exp_pool.tile([128, 512], BF16)
                            nc.scalar.activation(
                                ex,
                                ps_s,
                                mybir.ActivationFunctionType.Exp,
                                scale=1.0 / np.sqrt(HD),
                            )
                            di = kt - 4 * qc
                            if di >= 0:
                                nc.vector.tensor_mul(ex, ex, masks_sb[:, di, :])
                            if debug and mt == 0 and qc == 0 and kt == 0 and parity == 0:
                                dex = small_pool.tile([128, 512], F32, tag="dex")
                                nc.vector.tensor_copy(dex, ex)
                                nc.sync.dma_start(out=dbg["d_ex"][:, :], in_=dex)
                            lo, hi = V_BLK[2 * parity]  # 65 vs 128 wide block
                            blo, bhi = V_BLK[head]
                            m_width = bhi - blo
                            nc.tensor.matmul(
                                ps_u[parity][0:m_width, :],
                                lhsT=v_sb[:, kt, blo:bhi],
                                rhs=ex,
                                start=(kt == 0),
                                stop=(kt == nkt - 1),
                            )
                    for parity in (0, 1):
                        pofs = 64 * parity
                        r_part = 64 if parity == 0 else 0
                        data_lo = 0 if parity == 0 else 64
                        u = ps_u[parity]
                        rinv = small_pool.tile([128, 512], F32, tag="rinv")
                        nc.vector.reciprocal(
                            rinv[r_part : r_part + 1, :], u[r_part : r_part + 1, :]
                        )
                        rb = small_pool.tile([128, 512], F32, tag="rb")
                        ridx = (mt * QC + qc) * 2 + parity
                        nc.sync.dma_start(
                            out=rscr[ridx : ridx + 1, :],
                            in_=rinv[r_part : r_part + 1, :],
                        )
                        bsrc = bass.AP(
                            tensor=rscr.tensor,
                            offset=rscr[ridx : ridx + 1, :].offset,
                            ap=[[0, 64]] + list(rscr[ridx : ridx + 1, :].ap[1:]),
                        )
                        nc.gpsimd.dma_start(out=rb[pofs : pofs + 64, :], in_=bsrc)
                        if debug and mt == 0 and qc == 0 and parity == 0:
                            du = small_pool.tile([128, 512], F32, tag="du")
                            nc.vector.tensor_copy(du, u)
                            nc.sync.dma_start(out=dbg["d_u"][:, :], in_=du)
                            nc.sync.dma_start(out=dbg["d_rb"][:, :], in_=rb)
                        nc.vector.tensor_mul(
                            ctxT_sb[pofs : pofs + 64, mt, 512 * qc : 512 * (qc + 1)],
                            u[data_lo : data_lo + 64, :],
                            rb[pofs : pofs + 64, :],
                        )

            # ---- output projection: Y = ctx @ Wo_c ----
            for st in range(ST):
                for ncol in range(2):
                    ps = ps_mm.tile([128, 512], F32, tag="mm")
                    for kt2 in range(2):
                        nc.tensor.matmul(
                            ps,
                            lhsT=ctxT_sb[:, kt2, 128 * st : 128 * (st + 1)],
                            rhs=wo_sb[:, kt2, 512 * ncol : 512 * (ncol + 1)],
                            start=(kt2 == 0),
                            stop=(kt2 == 1),
                        )
                    ysb = io_pool.tile([128, 512], F32)
                    nc.vector.tensor_copy(ysb, ps)
                    nc.sync.dma_start(
                        out=y_r[st][:, 512 * ncol : 512 * (ncol + 1)], in_=ysb
                    )

            if debug:
                for nm, sb in (("d_qT", qT_sb), ("d_kT", kT_sb), ("d_v", v_sb), ("d_ctxT", ctxT_sb)):
                    flat = sb.rearrange("p a b -> p (a b)")
                    w = flat.shape[1]
                    for off in range(0, w, 512):
                        wid = min(512, w - off)
                        tmp2 = io_pool.tile([128, 512], F32, tag="dtmp", name="dtmp")
                        nc.vector.tensor_copy(tmp2[:, 0:wid], flat[:, off : off + wid])
                        nc.sync.dma_start(out=dbg[nm][:, off : off + wid], in_=tmp2[:, 0:wid])
    nc.finalize()
    return nc


_NC = None


def _get_nc():
    global _NC
    if _NC is None:
        _NC = _build_nc()
    return _NC


def kernel(x, Wq, Wk, Wv, Wo):
    x = np.asarray(x, dtype=np.float32)
    bf = ml_dtypes.bfloat16
    in_maps = []
    for c in range(NCORES):
        b, g = divmod(c, 4)
        sl = slice(g * DH, (g + 1) * DH)
        in_maps.append(
            {
                "xT": np.ascontiguousarray(x[b].T).astype(bf),
                "wq": np.ascontiguousarray(np.asarray(Wq)[:, sl]).astype(bf),
                "wk": np.ascontiguousarray(np.asarray(Wk)[:, sl]).astype(bf),
                "wv": np.ascontiguousarray(np.asarray(Wv)[:, sl]).astype(bf),
                "wo": np.ascontiguousarray(np.asarray(Wo)[sl, :]).astype(bf),
            }
        )
    global _last_in_maps
    _last_in_maps = in_maps
    res = run_bass_kernel_spmd(
        _get_nc(), in_maps, core_ids=list(range(NCORES)), trace=False
    )
    out = np.zeros((B, N, D), dtype=np.float32)
    for c in range(NCORES):
        out[c // 4] += res.results[c]["y"]
    return out


# revision 12
# speedup vs baseline: 1.2218x; 1.2218x over previous
"""Multi-head causal attention (B=2, N=2048, D=1024, H=16) on 8 TRN2 NeuronCores.

Sharding: data-parallel over batch (2) x tensor-parallel over head groups (4),
so each core handles one batch element and 4 heads (256 of the 1024 hidden
channels). Wq/Wk/Wv are column-sharded, Wo row-sharded; each core emits a
partial output [2048, 1024] that the host sums over the 4 head groups.

Per-core dataflow (all matmuls bf16 with fp32 PSUM accumulation):
  xT (pre-transposed on host)      [1024, 2048]
  Q^T = Wq_c^T x^T, K^T likewise   [256, 2048]   (head h at partition 64*(h%2), m-tile h//2)
  V   = x Wv_c                     [2048, 256]   stored per seq-tile with an
                                   appended ones column per head (the ones row
                                   of the U matmul accumulates the softmax
                                   denominator alongside the context)
  S^T               [128k, 1024]   both heads of a pair packed per k-tile: the
                                   even head (partitions 0-63 of K^T/Q^T, PE row
                                   strips 0-1) writes cols 0-511, the odd head
                                   (partitions 64-127, strips 2-3) cols 512-1023
                                   -> one exp() ACTIVATE covers both heads
  expS = exp(S^T/8); causal masking multiplies the 4 diagonal-crossing tiles
  by precomputed 0/1 masks (exp never overflows: |S/8| < ~4 at this scale)
  U = V_aug^T expS accumulated over k-tiles; the denominator row is partition
  64 (even head) / 0 (odd head); ctx^T = U[data] * bcast(1/r) where 1/r uses
  reciprocal_approx_fast and the partition-broadcast goes through a DRAM
  bounce (step-0 partition APs are only legal for DRAM sources, and the
  gpsimd partition_broadcast ucode is broken on this runtime).
  Y = ctx^T^T Wo_c                 [2048, 1024] fp32 partial out, emitted per
                                   q-chunk so the output projection overlaps
                                   the next chunk's attention.
"""

import sys

sys.path.insert(0, "/opt/trn_rl_repo")

import numpy as np
import ml_dtypes

import concourse.bass as bass
import concourse.bacc as bacc
import concourse.mybir as mybir
from concourse.tile import TileContext
from concourse.bass_utils import run_bass_kernel_spmd

BF16 = mybir.dt.bfloat16
F32 = mybir.dt.float32

B, N, D, H = 2, 2048, 1024, 16
HD = 64          # head dim
HPC = 4          # heads per core
DH = HPC * HD    # 256 hidden channels per core
NCORES = 8
KT = D // 128    # 8 contraction tiles over D
ST = N // 128    # 16 seq tiles
QC = N // 512    # 4 q-chunks of 512

# v_sb per-seq-tile column layout: for each head pair, an "even" block
# [V(64) | ones(1)] (matmul M=65 -> U partitions 0..64, denom at 64) and an
# "odd" block [ones(1) | zeros(63) | V(64)] (M=128 -> U partitions 64..127
# hold data, denom at partition 0, zeros keep partitions 1..63 inert).
V_BLK = {0: (0, 65), 1: (65, 193), 2: (193, 258), 3: (258, 386)}
V_COLS = 386
V_DATA_OFF = {0: 0, 1: 129, 2: 193, 3: 322}


def _build_nc(debug: bool = False) -> bass.Bass:
    nc = bacc.Bacc()
    xT = nc.declare_dram_parameter("xT", [D, N], BF16, isOutput=False)
    wq = nc.declare_dram_parameter("wq", [D, DH], BF16, isOutput=False)
    wk = nc.declare_dram_parameter("wk", [D, DH], BF16, isOutput=False)
    wv = nc.declare_dram_parameter("wv", [D, DH], BF16, isOutput=False)
    wo = nc.declare_dram_parameter("wo", [DH, D], BF16, isOutput=False)
    y = nc.declare_dram_parameter("y", [N, D], F32, isOutput=True)
    if debug:
        dbg = {
            "d_qT": nc.declare_dram_parameter("d_qT", [128, 2 * N], F32, isOutput=True),
            "d_kT": nc.declare_dram_parameter("d_kT", [128, 2 * N], F32, isOutput=True),
            "d_v": nc.declare_dram_parameter("d_v", [128, ST * V_COLS], F32, isOutput=True),
            "d_ctxT": nc.declare_dram_parameter("d_ctxT", [128, 2 * N], F32, isOutput=True),
        }

    xT_r = xT.rearrange("(t p) n -> t p n", p=128)
    wq_r = wq.rearrange("(t p) m -> t p m", p=128)
    wk_r = wk.rearrange("(t p) m -> t p m", p=128)
    wv_r = wv.rearrange("(t p) m -> t p m", p=128)
    wo_r = wo.rearrange("(t p) m -> t p m", p=128)
    y_r = y.rearrange("(t p) m -> t p m", p=128)

    with TileContext(nc) as tc:
        with (
            tc.tile_pool(name="const", bufs=1) as cpool,
            tc.tile_pool(name="io", bufs=3) as io_pool,
            tc.tile_pool(name="exps", bufs=6) as exp_pool,
            tc.tile_pool(name="small", bufs=4) as small_pool,
            tc.tile_pool(name="ps_big", bufs=2, space="PSUM") as ps_big,
            tc.tile_pool(name="ps_u", bufs=2, space="PSUM") as ps_u_pool,
            tc.tile_pool(name="drams", bufs=1, space="DRAM") as dram_pool,
        ):
            rscr = dram_pool.tile([16, 512], F32)
            xT_sb = cpool.tile([128, KT, N], BF16)
            wq_sb = cpool.tile([128, KT, DH], BF16)
            wk_sb = cpool.tile([128, KT, DH], BF16)
            wv_sb = cpool.tile([128, KT, DH], BF16)
            wo_sb = cpool.tile([128, 2, D], BF16)
            qT_sb = cpool.tile([128, 2, N], BF16)
            kT_sb = cpool.tile([128, 2, N], BF16)
            v_sb = cpool.tile([128, ST, V_COLS], BF16)
            ctxT_sb = cpool.tile([128, 2, N], BF16)
            masks_sb = cpool.tile([128, 4, 1024], BF16)

            for t in range(KT):
                nc.sync.dma_start(out=xT_sb[:, t, :], in_=xT_r[t])
                nc.sync.dma_start(out=wq_sb[:, t, :], in_=wq_r[t])
                nc.sync.dma_start(out=wk_sb[:, t, :], in_=wk_r[t])
                nc.sync.dma_start(out=wv_sb[:, t, :], in_=wv_r[t])
            for t in range(2):
                nc.sync.dma_start(out=wo_sb[:, t, :], in_=wo_r[t])

            # Causal masks for the 4 diagonal-crossing k-tiles of a q-chunk:
            # keep (1.0) where dq >= dk + 128*i, replicated in both halves so
            # one [128, 1024] multiply masks both packed heads.
            for i in range(4):
                nc.vector.memset(masks_sb[:, i, :], 1.0)
                m2 = masks_sb[:, i, :].rearrange("p (h q) -> p h q", q=512)
                nc.gpsimd.affine_select(
                    out=m2,
                    in_=m2,
                    compare_op=mybir.AluOpType.is_ge,
                    fill=0.0,
                    base=-(128 * i),
                    pattern=[[0, 2], [1, 512]],
                    channel_multiplier=-1,
                )

            # ones / zeros scaffolding of the V blocks (all seq tiles at once)
            nc.vector.memset(v_sb[:, :, 66:129], 0.0)
            nc.vector.memset(v_sb[:, :, 259:322], 0.0)
            for col in (64, 65, 257, 258):
                nc.vector.memset(v_sb[:, :, col : col + 1], 1.0)

            # ---- Q^T / K^T projections: [256, 2048] each, 2 q-chunks per PSUM ----
            for w_sb, dst in ((wq_sb, qT_sb), (wk_sb, kT_sb)):
                for mt in range(2):
                    for qcp in range(QC // 2):
                        ps = ps_big.tile([128, 1024], F32, tag="big", name="ps")
                        for kt in range(KT):
                            for half in range(2):
                                qc = 2 * qcp + half
                                nc.tensor.matmul(
                                    ps[:, 512 * half : 512 * (half + 1)],
                                    lhsT=w_sb[:, kt, 128 * mt : 128 * (mt + 1)],
                                    rhs=xT_sb[:, kt, 512 * qc : 512 * (qc + 1)],
                                    start=(kt == 0),
                                    stop=(kt == KT - 1),
                                )
                        nc.vector.tensor_copy(
                            dst[:, mt, 1024 * qcp : 1024 * (qcp + 1)], ps
                        )

            # ---- V = x @ Wv_c, stored per seq tile with ones columns ----
            for st in range(ST):
                ps = ps_big.tile([128, 1024], F32, tag="big", name="ps")
                psv = ps[:, 0:DH]
                for kt in range(KT):
                    nc.tensor.matmul(
                        psv,
                        lhsT=xT_sb[:, kt, 128 * st : 128 * (st + 1)],
                        rhs=wv_sb[:, kt, :],
                        start=(kt == 0),
                        stop=(kt == KT - 1),
                    )
                ps_h = ps.rearrange("p (h d) -> p h d", d=HD)
                # even heads 0,2 -> offsets 0,193; odd heads 1,3 -> 129,322
                ev = bass.AP(
                    tensor=v_sb.tensor,
                    offset=v_sb[:, st, 0:1].offset,
                    ap=[v_sb.ap[0], [193, 2], [1, HD]],
                )
                od = bass.AP(
                    tensor=v_sb.tensor,
                    offset=v_sb[:, st, 129:130].offset,
                    ap=[v_sb.ap[0], [193, 2], [1, HD]],
                )
                in_ev = bass.AP(
                    tensor=ps.tensor,
                    offset=ps_h[:, 0, :].offset,
                    ap=[ps.ap[0], [2 * HD, 2], [1, HD]],
                )
                in_od = bass.AP(
                    tensor=ps.tensor,
                    offset=ps_h[:, 1, :].offset,
                    ap=[ps.ap[0], [2 * HD, 2], [1, HD]],
                )
                nc.vector.tensor_copy(ev, in_ev)
                nc.vector.tensor_copy(od, in_od)

            # ---- attention + output projection, interleaved per q-chunk ----
            for qc in range(QC):
                nkt = 4 * (qc + 1)
                for mt in range(2):
                    ps_u = {
                        0: ps_u_pool.tile([128, 512], F32, tag="ue", name="ue"),
                        1: ps_u_pool.tile([128, 512], F32, tag="uo", name="uo"),
                    }
                    for kt in range(nkt):
                        # S^T for both heads of the pair into one 2-bank tile
                        ps_s = ps_big.tile([128, 1024], F32, tag="big", name="ps_s")
                        for parity in (0, 1):
                            pofs = 64 * parity
                            nc.tensor.matmul(
                                ps_s[:, 512 * parity : 512 * (parity + 1)],
                                lhsT=kT_sb[
                                    pofs : pofs + 64, mt, 128 * kt : 128 * (kt + 1)
                                ],
                                rhs=qT_sb[
                                    pofs : pofs + 64, mt, 512 * qc : 512 * (qc + 1)
                                ],
                                start=True,
                                stop=True,
                            )
                        ex = exp_pool.tile([128, 1024], BF16)
                        nc.scalar.activation(
                            ex,
                            ps_s,
                            mybir.ActivationFunctionType.Exp,
                            scale=1.0 / np.sqrt(HD),
                        )
                        di = kt - 4 * qc
                        if di >= 0:
                            nc.vector.tensor_mul(ex, ex, masks_sb[:, di, :])
                        for parity in (0, 1):
                            head = 2 * mt + parity
                            blo, bhi = V_BLK[head]
                            nc.tensor.matmul(
                                ps_u[parity][0 : bhi - blo, :],
                                lhsT=v_sb[:, kt, blo:bhi],
                                rhs=ex[:, 512 * parity : 512 * (parity + 1)],
                                start=(kt == 0),
                                stop=(kt == nkt - 1),
                            )
                    for parity in (0, 1):
                        # reciprocal_approx_fast (custom DVE ucode) only works
                        # on APs based at partition 0 on this runtime: the odd
                        # head (denom at partition 0) takes 1/r before the
                        # DRAM-bounce broadcast; the even head (denom at
                        # partition 64) broadcasts raw r to partitions 0-63
                        # first and takes the reciprocal there.
                        pofs = 64 * parity
                        r_part = 64 if parity == 0 else 0
                        data_lo = 0 if parity == 0 else 64
                        u = ps_u[parity]
                        ridx = (mt * QC + qc) * 2 + parity
                        rinv = small_pool.tile([128, 512], F32, tag="rinv")
                        if parity == 1:
                            nc.vector.reciprocal_approx_fast(
                                out=rinv[0:1, :], in_=u[0:1, :]
                            )
                            src = rinv[0:1, :]
                        else:
                            nc.vector.tensor_copy(rinv[64:65, :], u[64:65, :])
                            src = rinv[64:65, :]
                        nc.sync.dma_start(out=rscr[ridx : ridx + 1, :], in_=src)
                        rb = small_pool.tile([128, 512], F32, tag="rb")
                        bsrc = bass.AP(
                            tensor=rscr.tensor,
                            offset=rscr[ridx : ridx + 1, :].offset,
                            ap=[[0, 64]] + list(rscr[ridx : ridx + 1, :].ap[1:]),
                        )
                        nc.gpsimd.dma_start(out=rb[pofs : pofs + 64, :], in_=bsrc)
                        if parity == 0:
                            nc.vector.reciprocal_approx_fast(
                                out=rb[0:64, :], in_=rb[0:64, :]
                            )
                        nc.vector.tensor_mul(
                            ctxT_sb[pofs : pofs + 64, mt, 512 * qc : 512 * (qc + 1)],
                            u[data_lo : data_lo + 64, :],
                            rb[pofs : pofs + 64, :],
                        )

                # output projection for this q-chunk's 4 seq tiles
                for st in range(4 * qc, 4 * qc + 4):
                    ps = ps_big.tile([128, 1024], F32, tag="big", name="ps")
                    for half in range(2):
                        for kt2 in range(2):
                            nc.tensor.matmul(
                                ps[:, 512 * half : 512 * (half + 1)],
                                lhsT=ctxT_sb[:, kt2, 128 * st : 128 * (st + 1)],
                                rhs=wo_sb[:, kt2, 512 * half : 512 * (half + 1)],
                                start=(kt2 == 0),
                                stop=(kt2 == 1),
                            )
                    ysb = io_pool.tile([128, 1024], F32)
                    nc.vector.tensor_copy(ysb, ps)
                    nc.sync.dma_start(out=y_r[st], in_=ysb)

            if debug:
                for nm, sb in (
                    ("d_qT", qT_sb),
                    ("d_kT", kT_sb),
                    ("d_v", v_sb),
                    ("d_ctxT", ctxT_sb),
                ):
                    flat = sb.rearrange("p a b -> p (a b)")
                    w = flat.shape[1]
                    for off in range(0, w, 512):
                        wid = min(512, w - off)
                        tmp2 = io_pool.tile([128, 1024], F32, tag="dtmp", name="dtmp")
                        nc.vector.tensor_copy(tmp2[:, 0:wid], flat[:, off : off + wid])
                        nc.sync.dma_start(
                            out=dbg[nm][:, off : off + wid], in_=tmp2[:, 0:wid]
                        )
    nc.finalize()
    return nc


_NC = None


def _get_nc():
    global _NC
    if _NC is None:
        _NC = _build_nc()
    return _NC


def kernel(x, Wq, Wk, Wv, Wo):
    x = np.asarray(x, dtype=np.float32)
    bf = ml_dtypes.bfloat16
    in_maps = []
    for c in range(NCORES):
        b, g = divmod(c, 4)
        sl = slice(g * DH, (g + 1) * DH)
        in_maps.append(
            {
                "xT": np.ascontiguousarray(x[b].T).astype(bf),
                "wq": np.ascontiguousarray(np.asarray(Wq)[:, sl]).astype(bf),
                "wk": np.ascontiguousarray(np.asarray(Wk)[:, sl]).astype(bf),
                "wv": np.ascontiguousarray(np.asarray(Wv)[:, sl]).astype(bf),
                "wo": np.ascontiguousarray(np.asarray(Wo)[sl, :]).astype(bf),
            }
        )
    global _last_in_maps
    _last_in_maps = in_maps
    res = run_bass_kernel_spmd(
        _get_nc(), in_maps, core_ids=list(range(NCORES)), trace=False
    )
    out = np.zeros((B, N, D), dtype=np.float32)
    for c in range(NCORES):
        out[c // 4] += res.results[c]["y"]
    return out


# revision 14
# speedup vs baseline: 1.4119x; 1.1556x over previous
"""Multi-head causal attention (B=2, N=2048, D=1024, H=16) on 8 TRN2 NeuronCores.

Sharding: data-parallel over batch (2) x tensor-parallel over head groups (4),
so each core handles one batch element and 4 heads (256 of the 1024 hidden
channels). Wq/Wk/Wv are column-sharded, Wo row-sharded; each core emits a
partial output [2048, 1024] that the host sums over the 4 head groups.

Per-core dataflow (all matmuls bf16 with fp32 PSUM accumulation):
  xT (pre-transposed on host)      [1024, 2048]
  Q^T = Wq_c^T x^T, K^T likewise   [256, 2048]   (head h at partition 64*(h%2), m-tile h//2)
  V   = x Wv_c                     [2048, 256]   stored per seq-tile with an
                                   appended ones column per head (the ones row
                                   of the U matmul accumulates the softmax
                                   denominator alongside the context)
  S^T               [128k, 1024]   both heads of a pair packed per k-tile: the
                                   even head (partitions 0-63 of K^T/Q^T, PE row
                                   strips 0-1) writes cols 0-511, the odd head
                                   (partitions 64-127, strips 2-3) cols 512-1023
                                   -> one exp() ACTIVATE covers both heads
  expS = exp(S^T/8); causal masking multiplies the 4 diagonal-crossing tiles
  by precomputed 0/1 masks (exp never overflows: |S/8| < ~4 at this scale)
  U = V_aug^T expS accumulated over k-tiles; the denominator row is partition
  64 (even head) / 0 (odd head); ctx^T = U[data] * bcast(1/r) where 1/r uses
  reciprocal_approx_fast and the partition-broadcast goes through a DRAM
  bounce (step-0 partition APs are only legal for DRAM sources, and the
  gpsimd partition_broadcast ucode is broken on this runtime).
  Y = ctx^T^T Wo_c                 [2048, 1024] fp32 partial out, emitted per
                                   q-chunk so the output projection overlaps
                                   the next chunk's attention.
"""

import sys

sys.path.insert(0, "/opt/trn_rl_repo")

import numpy as np
import ml_dtypes

import concourse.bass as bass
import concourse.bacc as bacc
import concourse.mybir as mybir
from concourse.tile import TileContext
from concourse.bass_utils import run_bass_kernel_spmd

BF16 = mybir.dt.bfloat16
F32 = mybir.dt.float32

B, N, D, H = 2, 2048, 1024, 16
HD = 64          # head dim
HPC = 4          # heads per core
DH = HPC * HD    # 256 hidden channels per core
NCORES = 8
KT = D // 128    # 8 contraction tiles over D
ST = N // 128    # 16 seq tiles
QC = N // 512    # 4 q-chunks of 512

# v_sb per-seq-tile column layout: for each head pair, an "even" block
# [V(64) | ones(1)] (matmul M=65 -> U partitions 0..64, denom at 64) and an
# "odd" block [ones(1) | zeros(63) | V(64)] (M=128 -> U partitions 64..127
# hold data, denom at partition 0, zeros keep partitions 1..63 inert).
V_BLK = {0: (0, 65), 1: (65, 193), 2: (193, 258), 3: (258, 386)}
V_COLS = 386
V_DATA_OFF = {0: 0, 1: 129, 2: 193, 3: 322}


def _y_tiles_for_iteration(qc):
    # Y seq-tiles emitted during attention iteration qc: chunk qc-1's tiles
    # while qc < QC runs, plus chunk QC-1's own tiles at the end.
    tiles = []
    if qc > 0:
        tiles += list(range(4 * (qc - 1), 4 * qc))
    if qc == QC - 1:
        tiles += list(range(4 * qc, 4 * (qc + 1)))
    return tiles


def _build_nc(debug: bool = False) -> bass.Bass:
    nc = bacc.Bacc()
    xT = nc.declare_dram_parameter("xT", [D, N], BF16, isOutput=False)
    wq = nc.declare_dram_parameter("wq", [D, DH], BF16, isOutput=False)
    wk = nc.declare_dram_parameter("wk", [D, DH], BF16, isOutput=False)
    wv = nc.declare_dram_parameter("wv", [D, DH], BF16, isOutput=False)
    wo = nc.declare_dram_parameter("wo", [DH, D], BF16, isOutput=False)
    y = nc.declare_dram_parameter("y", [N, D], F32, isOutput=True)
    if debug:
        dbg = {
            "d_qT": nc.declare_dram_parameter("d_qT", [128, 2 * N], F32, isOutput=True),
            "d_kT": nc.declare_dram_parameter("d_kT", [128, 2 * N], F32, isOutput=True),
            "d_v": nc.declare_dram_parameter("d_v", [128, ST * V_COLS], F32, isOutput=True),
            "d_ctxT": nc.declare_dram_parameter("d_ctxT", [128, 2 * N], F32, isOutput=True),
        }

    xT_r = xT.rearrange("(t p) n -> t p n", p=128)
    wq_r = wq.rearrange("(t p) m -> t p m", p=128)
    wk_r = wk.rearrange("(t p) m -> t p m", p=128)
    wv_r = wv.rearrange("(t p) m -> t p m", p=128)
    wo_r = wo.rearrange("(t p) m -> t p m", p=128)
    y_r = y.rearrange("(t p) m -> t p m", p=128)

    with TileContext(nc) as tc:
        with (
            tc.tile_pool(name="const", bufs=1) as cpool,
            tc.tile_pool(name="io", bufs=3) as io_pool,
            tc.tile_pool(name="exps", bufs=6) as exp_pool,
            tc.tile_pool(name="small", bufs=4) as small_pool,
            tc.tile_pool(name="ps_big", bufs=2, space="PSUM") as ps_big,
            tc.tile_pool(name="ps_u", bufs=2, space="PSUM") as ps_u_pool,
            tc.tile_pool(name="drams", bufs=1, space="DRAM") as dram_pool,
        ):
            rscr = dram_pool.tile([16, 512], F32)
            xT_sb = cpool.tile([128, KT, N], BF16)
            wq_sb = cpool.tile([128, KT, DH], BF16)
            wk_sb = cpool.tile([128, KT, DH], BF16)
            wv_sb = cpool.tile([128, KT, DH], BF16)
            wo_sb = cpool.tile([128, 2, D], BF16)
            qT_sb = cpool.tile([128, 2, N], BF16)
            kT_sb = cpool.tile([128, 2, N], BF16)
            v_sb = cpool.tile([128, ST, V_COLS], BF16)
            ctxT_sb = cpool.tile([128, 2, N], BF16)
            masks_sb = cpool.tile([128, 4, 1024], BF16)

            for t in range(KT):
                nc.sync.dma_start(out=xT_sb[:, t, :], in_=xT_r[t])
                nc.sync.dma_start(out=wq_sb[:, t, :], in_=wq_r[t])
                nc.sync.dma_start(out=wk_sb[:, t, :], in_=wk_r[t])
                nc.sync.dma_start(out=wv_sb[:, t, :], in_=wv_r[t])
            for t in range(2):
                nc.sync.dma_start(out=wo_sb[:, t, :], in_=wo_r[t])

            # Causal masks for the 4 diagonal-crossing k-tiles of a q-chunk:
            # keep (1.0) where dq >= dk + 128*i, replicated in both halves so
            # one [128, 1024] multiply masks both packed heads.
            for i in range(4):
                nc.vector.memset(masks_sb[:, i, :], 1.0)
                m2 = masks_sb[:, i, :].rearrange("p (h q) -> p h q", q=512)
                nc.gpsimd.affine_select(
                    out=m2,
                    in_=m2,
                    compare_op=mybir.AluOpType.is_ge,
                    fill=0.0,
                    base=-(128 * i),
                    pattern=[[0, 2], [1, 512]],
                    channel_multiplier=-1,
                )

            # ones / zeros scaffolding of the V blocks (all seq tiles at once)
            nc.vector.memset(v_sb[:, :, 66:129], 0.0)
            nc.vector.memset(v_sb[:, :, 259:322], 0.0)
            for col in (64, 65, 257, 258):
                nc.vector.memset(v_sb[:, :, col : col + 1], 1.0)

            # ---- Q^T / K^T projections: [256, 2048] each, 2 q-chunks per PSUM ----
            for w_sb, dst in ((wq_sb, qT_sb), (wk_sb, kT_sb)):
                for mt in range(2):
                    for qcp in range(QC // 2):
                        ps = ps_big.tile([128, 1024], F32, tag="big", name="ps")
                        for kt in range(KT):
                            for half in range(2):
                                qc = 2 * qcp + half
                                nc.tensor.matmul(
                                    ps[:, 512 * half : 512 * (half + 1)],
                                    lhsT=w_sb[:, kt, 128 * mt : 128 * (mt + 1)],
                                    rhs=xT_sb[:, kt, 512 * qc : 512 * (qc + 1)],
                                    start=(kt == 0),
                                    stop=(kt == KT - 1),
                                )
                        nc.vector.tensor_copy(
                            dst[:, mt, 1024 * qcp : 1024 * (qcp + 1)], ps
                        )

            # ---- V = x @ Wv_c, stored per seq tile with ones columns ----
            for st in range(ST):
                ps = ps_big.tile([128, 1024], F32, tag="big", name="ps")
                psv = ps[:, 0:DH]
                for kt in range(KT):
                    nc.tensor.matmul(
                        psv,
                        lhsT=xT_sb[:, kt, 128 * st : 128 * (st + 1)],
                        rhs=wv_sb[:, kt, :],
                        start=(kt == 0),
                        stop=(kt == KT - 1),
                    )
                ps_h = ps.rearrange("p (h d) -> p h d", d=HD)
                # even heads 0,2 -> offsets 0,193; odd heads 1,3 -> 129,322
                ev = bass.AP(
                    tensor=v_sb.tensor,
                    offset=v_sb[:, st, 0:1].offset,
                    ap=[v_sb.ap[0], [193, 2], [1, HD]],
                )
                od = bass.AP(
                    tensor=v_sb.tensor,
                    offset=v_sb[:, st, 129:130].offset,
                    ap=[v_sb.ap[0], [193, 2], [1, HD]],
                )
                in_ev = bass.AP(
                    tensor=ps.tensor,
                    offset=ps_h[:, 0, :].offset,
                    ap=[ps.ap[0], [2 * HD, 2], [1, HD]],
                )
                in_od = bass.AP(
                    tensor=ps.tensor,
                    offset=ps_h[:, 1, :].offset,
                    ap=[ps.ap[0], [2 * HD, 2], [1, HD]],
                )
                nc.vector.tensor_copy(ev, in_ev)
                nc.vector.tensor_copy(od, in_od)

            # ---- attention + output projection, interleaved per q-chunk ----
            for qc in range(QC):
                nkt = 4 * (qc + 1)
                for mt in range(2):
                    ps_u = {
                        0: ps_u_pool.tile([128, 512], F32, tag="ue", name="ue"),
                        1: ps_u_pool.tile([128, 512], F32, tag="uo", name="uo"),
                    }
                    for kt in range(nkt):
                        # S^T for both heads of the pair into one 2-bank tile
                        ps_s = ps_big.tile([128, 1024], F32, tag="big", name="ps_s")
                        for parity in (0, 1):
                            pofs = 64 * parity
                            nc.tensor.matmul(
                                ps_s[:, 512 * parity : 512 * (parity + 1)],
                                lhsT=kT_sb[
                                    pofs : pofs + 64, mt, 128 * kt : 128 * (kt + 1)
                                ],
                                rhs=qT_sb[
                                    pofs : pofs + 64, mt, 512 * qc : 512 * (qc + 1)
                                ],
                                start=True,
                                stop=True,
                            )
                        ex = exp_pool.tile([128, 1024], BF16)
                        nc.scalar.activation(
                            ex,
                            ps_s,
                            mybir.ActivationFunctionType.Exp,
                            scale=1.0 / np.sqrt(HD),
                        )
                        di = kt - 4 * qc
                        if di >= 0:
                            nc.vector.tensor_mul(ex, ex, masks_sb[:, di, :])
                        for parity in (0, 1):
                            head = 2 * mt + parity
                            blo, bhi = V_BLK[head]
                            nc.tensor.matmul(
                                ps_u[parity][0 : bhi - blo, :],
                                lhsT=v_sb[:, kt, blo:bhi],
                                rhs=ex[:, 512 * parity : 512 * (parity + 1)],
                                start=(kt == 0),
                                stop=(kt == nkt - 1),
                            )
                    for parity in (0, 1):
                        # reciprocal_approx_fast (custom DVE ucode) only works
                        # on APs based at partition 0 on this runtime: the odd
                        # head (denom at partition 0) takes 1/r before the
                        # DRAM-bounce broadcast; the even head (denom at
                        # partition 64) broadcasts raw r to partitions 0-63
                        # first and takes the reciprocal there.
                        pofs = 64 * parity
                        r_part = 64 if parity == 0 else 0
                        data_lo = 0 if parity == 0 else 64
                        u = ps_u[parity]
                        ridx = (mt * QC + qc) * 2 + parity
                        rinv = small_pool.tile([128, 512], F32, tag="rinv")
                        if parity == 1:
                            nc.vector.reciprocal_approx_fast(
                                out=rinv[0:1, :], in_=u[0:1, :]
                            )
                            src = rinv[0:1, :]
                        else:
                            nc.vector.tensor_copy(rinv[64:65, :], u[64:65, :])
                            src = rinv[64:65, :]
                        nc.sync.dma_start(out=rscr[ridx : ridx + 1, :], in_=src)
                        rb = small_pool.tile([128, 512], F32, tag="rb")
                        bsrc = bass.AP(
                            tensor=rscr.tensor,
                            offset=rscr[ridx : ridx + 1, :].offset,
                            ap=[[0, 64]] + list(rscr[ridx : ridx + 1, :].ap[1:]),
                        )
                        nc.gpsimd.dma_start(out=rb[pofs : pofs + 64, :], in_=bsrc)
                        if parity == 0:
                            nc.vector.reciprocal_approx_fast(
                                out=rb[0:64, :], in_=rb[0:64, :]
                            )
                        nc.vector.tensor_mul(
                            ctxT_sb[pofs : pofs + 64, mt, 512 * qc : 512 * (qc + 1)],
                            u[data_lo : data_lo + 64, :],
                            rb[pofs : pofs + 64, :],
                        )

                # output projection runs one q-chunk behind the attention so
                # the in-order PE queue never stalls on the normalize chain
                # (U -> reciprocal -> DRAM-bounce broadcast -> ctx multiply).
                for st in _y_tiles_for_iteration(qc):
                    ps = ps_big.tile([128, 1024], F32, tag="big", name="ps")
                    for half in range(2):
                        for kt2 in range(2):
                            nc.tensor.matmul(
                                ps[:, 512 * half : 512 * (half + 1)],
                                lhsT=ctxT_sb[:, kt2, 128 * st : 128 * (st + 1)],
                                rhs=wo_sb[:, kt2, 512 * half : 512 * (half + 1)],
                                start=(kt2 == 0),
                                stop=(kt2 == 1),
                            )
                    ysb = io_pool.tile([128, 1024], F32)
                    nc.vector.tensor_copy(ysb, ps)
                    nc.sync.dma_start(out=y_r[st], in_=ysb)

            if debug:
                for nm, sb in (
                    ("d_qT", qT_sb),
                    ("d_kT", kT_sb),
                    ("d_v", v_sb),
                    ("d_ctxT", ctxT_sb),
                ):
                    flat = sb.rearrange("p a b -> p (a b)")
                    w = flat.shape[1]
                    for off in range(0, w, 512):
                        wid = min(512, w - off)
                        tmp2 = io_pool.tile([128, 1024], F32, tag="dtmp", name="dtmp")
                        nc.vector.tensor_copy(tmp2[:, 0:wid], flat[:, off : off + wid])
                        nc.sync.dma_start(
                            out=dbg[nm][:, off : off + wid], in_=tmp2[:, 0:wid]
                        )
    nc.finalize()
    return nc


_NC = None


def _get_nc():
    global _NC
    if _NC is None:
        _NC = _build_nc()
    return _NC


def kernel(x, Wq, Wk, Wv, Wo):
    x = np.asarray(x, dtype=np.float32)
    bf = ml_dtypes.bfloat16
    in_maps = []
    for c in range(NCORES):
        b, g = divmod(c, 4)
        sl = slice(g * DH, (g + 1) * DH)
        in_maps.append(
            {
                "xT": np.ascontiguousarray(x[b].T).astype(bf),
                "wq": np.ascontiguousarray(np.asarray(Wq)[:, sl]).astype(bf),
                "wk": np.ascontiguousarray(np.asarray(Wk)[:, sl]).astype(bf),
                "wv": np.ascontiguousarray(np.asarray(Wv)[:, sl]).astype(bf),
                "wo": np.ascontiguousarray(np.asarray(Wo)[sl, :]).astype(bf),
            }
        )
    global _last_in_maps
    _last_in_maps = in_maps
    res = run_bass_kernel_spmd(
        _get_nc(), in_maps, core_ids=list(range(NCORES)), trace=False
    )
    out = np.zeros((B, N, D), dtype=np.float32)
    for c in range(NCORES):
        out[c // 4] += res.results[c]["y"]
    return out


# revision 16
# speedup vs baseline: 1.4447x; 1.0232x over previous
"""Multi-head causal attention (B=2, N=2048, D=1024, H=16) on 8 TRN2 NeuronCores.

Sharding: data-parallel over batch (2) x tensor-parallel over head groups (4),
so each core handles one batch element and 4 heads (256 of the 1024 hidden
channels). Wq/Wk/Wv are column-sharded, Wo row-sharded; each core emits a
partial output [2048, 1024] that the host sums over the 4 head groups.

Per-core dataflow (all matmuls bf16 with fp32 PSUM accumulation):
  xT (pre-transposed on host)      [1024, 2048]
  Q^T = Wq_c^T x^T, K^T likewise   [256, 2048]   (head h at partition 64*(h%2), m-tile h//2)
  V   = x Wv_c                     [2048, 256]   stored per seq-tile with an
                                   appended ones column per head (the ones row
                                   of the U matmul accumulates the softmax
                                   denominator alongside the context)
  S^T               [128k, 1024]   both heads of a pair packed per k-tile: the
                                   even head (partitions 0-63 of K^T/Q^T, PE row
                                   strips 0-1) writes cols 0-511, the odd head
                                   (partitions 64-127, strips 2-3) cols 512-1023
                                   -> one exp() ACTIVATE covers both heads
  expS = exp(S^T/8); causal masking multiplies the 4 diagonal-crossing tiles
  by precomputed 0/1 masks (exp never overflows: |S/8| < ~4 at this scale)
  U = V_aug^T expS accumulated over k-tiles; the denominator row is partition
  64 (even head) / 0 (odd head); ctx^T = U[data] * bcast(1/r) where 1/r uses
  reciprocal_approx_fast and the partition-broadcast goes through a DRAM
  bounce (step-0 partition APs are only legal for DRAM sources, and the
  gpsimd partition_broadcast ucode is broken on this runtime).
  Y = ctx^T^T Wo_c                 [2048, 1024] fp32 partial out, emitted per
                                   q-chunk so the output projection overlaps
                                   the next chunk's attention.
"""

import sys

sys.path.insert(0, "/opt/trn_rl_repo")

import numpy as np
import ml_dtypes

import concourse.bass as bass
import concourse.bacc as bacc
import concourse.mybir as mybir
from concourse.tile import TileContext
from concourse.bass_utils import run_bass_kernel_spmd

BF16 = mybir.dt.bfloat16
F32 = mybir.dt.float32

B, N, D, H = 2, 2048, 1024, 16
HD = 64          # head dim
HPC = 4          # heads per core
DH = HPC * HD    # 256 hidden channels per core
NCORES = 8
KT = D // 128    # 8 contraction tiles over D
ST = N // 128    # 16 seq tiles
QC = N // 512    # 4 q-chunks of 512

# v_sb per-seq-tile column layout: for each head pair, an "even" block
# [V(64) | ones(1)] (matmul M=65 -> U partitions 0..64, denom at 64) and an
# "odd" block [ones(1) | zeros(63) | V(64)] (M=128 -> U partitions 64..127
# hold data, denom at partition 0, zeros keep partitions 1..63 inert).
V_BLK = {0: (0, 65), 1: (65, 193), 2: (193, 258), 3: (258, 386)}
V_COLS = 386
V_DATA_OFF = {0: 0, 1: 129, 2: 193, 3: 322}


def _y_tiles_for_iteration(qc):
    # Y seq-tiles emitted during attention iteration qc: chunk qc-1's tiles
    # while qc < QC runs, plus chunk QC-1's own tiles at the end.
    tiles = []
    if qc > 0:
        tiles += list(range(4 * (qc - 1), 4 * qc))
    if qc == QC - 1:
        tiles += list(range(4 * qc, 4 * (qc + 1)))
    return tiles


def _build_nc(debug: bool = False) -> bass.Bass:
    nc = bacc.Bacc()
    xT = nc.declare_dram_parameter("xT", [D, N], BF16, isOutput=False)
    wq = nc.declare_dram_parameter("wq", [D, DH], BF16, isOutput=False)
    wk = nc.declare_dram_parameter("wk", [D, DH], BF16, isOutput=False)
    wv = nc.declare_dram_parameter("wv", [D, DH], BF16, isOutput=False)
    wo = nc.declare_dram_parameter("wo", [DH, D], BF16, isOutput=False)
    y = nc.declare_dram_parameter("y", [N, D], F32, isOutput=True)
    if debug:
        dbg = {
            "d_qT": nc.declare_dram_parameter("d_qT", [128, 2 * N], F32, isOutput=True),
            "d_kT": nc.declare_dram_parameter("d_kT", [128, 2 * N], F32, isOutput=True),
            "d_v": nc.declare_dram_parameter("d_v", [128, ST * V_COLS], F32, isOutput=True),
            "d_ctxT": nc.declare_dram_parameter("d_ctxT", [128, 2 * N], F32, isOutput=True),
        }

    xT_r = xT.rearrange("(t p) n -> t p n", p=128)
    wq_r = wq.rearrange("(t p) m -> t p m", p=128)
    wk_r = wk.rearrange("(t p) m -> t p m", p=128)
    wv_r = wv.rearrange("(t p) m -> t p m", p=128)
    wo_r = wo.rearrange("(t p) m -> t p m", p=128)
    y_r = y.rearrange("(t p) m -> t p m", p=128)

    with TileContext(nc) as tc:
        with (
            tc.tile_pool(name="const", bufs=1) as cpool,
            tc.tile_pool(name="io", bufs=3) as io_pool,
            tc.tile_pool(name="exps", bufs=6) as exp_pool,
            tc.tile_pool(name="small", bufs=4) as small_pool,
            tc.tile_pool(name="ps_big", bufs=2, space="PSUM") as ps_big,
            tc.tile_pool(name="ps_u", bufs=2, space="PSUM") as ps_u_pool,
            tc.tile_pool(name="drams", bufs=1, space="DRAM") as dram_pool,
        ):
            rscr = dram_pool.tile([16, 512], F32)
            xT_sb = cpool.tile([128, KT, N], BF16)
            wq_sb = cpool.tile([128, KT, DH], BF16)
            wk_sb = cpool.tile([128, KT, DH], BF16)
            wv_sb = cpool.tile([128, KT, DH], BF16)
            wo_sb = cpool.tile([128, 2, D], BF16)
            qT_sb = cpool.tile([128, 2, N], BF16)
            kT_sb = cpool.tile([128, 2, N], BF16)
            v_sb = cpool.tile([128, ST, V_COLS], BF16)
            ctxT_sb = cpool.tile([128, 2, N], BF16)
            masks_sb = cpool.tile([128, 4, 1024], BF16)

            # xT and wq gate the first matmuls — land them first
            for t in range(KT):
                nc.sync.dma_start(out=xT_sb[:, t, :], in_=xT_r[t])
                nc.sync.dma_start(out=wq_sb[:, t, :], in_=wq_r[t])
            for t in range(KT):
                nc.sync.dma_start(out=wk_sb[:, t, :], in_=wk_r[t])
                nc.sync.dma_start(out=wv_sb[:, t, :], in_=wv_r[t])
            for t in range(2):
                nc.sync.dma_start(out=wo_sb[:, t, :], in_=wo_r[t])

            # Causal masks for the 4 diagonal-crossing k-tiles of a q-chunk:
            # keep (1.0) where dq >= dk + 128*i, replicated in both halves so
            # one [128, 1024] multiply masks both packed heads.
            for i in range(4):
                nc.vector.memset(masks_sb[:, i, :], 1.0)
                m2 = masks_sb[:, i, :].rearrange("p (h q) -> p h q", q=512)
                nc.gpsimd.affine_select(
                    out=m2,
                    in_=m2,
                    compare_op=mybir.AluOpType.is_ge,
                    fill=0.0,
                    base=-(128 * i),
                    pattern=[[0, 2], [1, 512]],
                    channel_multiplier=-1,
                )

            # ones / zeros scaffolding of the V blocks (all seq tiles at once)
            nc.vector.memset(v_sb[:, :, 66:129], 0.0)
            nc.vector.memset(v_sb[:, :, 259:322], 0.0)
            for col in (64, 65, 257, 258):
                nc.vector.memset(v_sb[:, :, col : col + 1], 1.0)

            # ---- Q^T / K^T projections: [256, 2048] each, 2 q-chunks per PSUM ----
            for w_sb, dst in ((wq_sb, qT_sb), (wk_sb, kT_sb)):
                for mt in range(2):
                    for qcp in range(QC // 2):
                        ps = ps_big.tile([128, 1024], F32, tag="big", name="ps")
                        for kt in range(KT):
                            for half in range(2):
                                qc = 2 * qcp + half
                                nc.tensor.matmul(
                                    ps[:, 512 * half : 512 * (half + 1)],
                                    lhsT=w_sb[:, kt, 128 * mt : 128 * (mt + 1)],
                                    rhs=xT_sb[:, kt, 512 * qc : 512 * (qc + 1)],
                                    start=(kt == 0),
                                    stop=(kt == KT - 1),
                                )
                        nc.vector.tensor_copy(
                            dst[:, mt, 1024 * qcp : 1024 * (qcp + 1)], ps
                        )

            # ---- V = x @ Wv_c, stored per seq tile with ones columns ----
            for st in range(ST):
                ps = ps_big.tile([128, 1024], F32, tag="big", name="ps")
                psv = ps[:, 0:DH]
                for kt in range(KT):
                    nc.tensor.matmul(
                        psv,
                        lhsT=xT_sb[:, kt, 128 * st : 128 * (st + 1)],
                        rhs=wv_sb[:, kt, :],
                        start=(kt == 0),
                        stop=(kt == KT - 1),
                    )
                ps_h = ps.rearrange("p (h d) -> p h d", d=HD)
                # even heads 0,2 -> offsets 0,193; odd heads 1,3 -> 129,322
                ev = bass.AP(
                    tensor=v_sb.tensor,
                    offset=v_sb[:, st, 0:1].offset,
                    ap=[v_sb.ap[0], [193, 2], [1, HD]],
                )
                od = bass.AP(
                    tensor=v_sb.tensor,
                    offset=v_sb[:, st, 129:130].offset,
                    ap=[v_sb.ap[0], [193, 2], [1, HD]],
                )
                in_ev = bass.AP(
                    tensor=ps.tensor,
                    offset=ps_h[:, 0, :].offset,
                    ap=[ps.ap[0], [2 * HD, 2], [1, HD]],
                )
                in_od = bass.AP(
                    tensor=ps.tensor,
                    offset=ps_h[:, 1, :].offset,
                    ap=[ps.ap[0], [2 * HD, 2], [1, HD]],
                )
                nc.vector.tensor_copy(ev, in_ev)
                nc.vector.tensor_copy(od, in_od)

            # ---- attention + output projection, interleaved per q-chunk ----
            for qc in range(QC):
                nkt = 4 * (qc + 1)
                for mt in range(2):
                    ps_u = {
                        0: ps_u_pool.tile([128, 512], F32, tag="ue", name="ue"),
                        1: ps_u_pool.tile([128, 512], F32, tag="uo", name="uo"),
                    }
                    def _pv(ex_prev, kt_prev):
                        for parity in (0, 1):
                            head = 2 * mt + parity
                            blo, bhi = V_BLK[head]
                            nc.tensor.matmul(
                                ps_u[parity][0 : bhi - blo, :],
                                lhsT=v_sb[:, kt_prev, blo:bhi],
                                rhs=ex_prev[:, 512 * parity : 512 * (parity + 1)],
                                start=(kt_prev == 0),
                                stop=(kt_prev == nkt - 1),
                            )

                    # PV lags one k-tile behind S so the in-order PE queue
                    # never has a PV (waiting on exp) ahead of ready S matmuls
                    prev = None
                    for kt in range(nkt):
                        # S^T for both heads of the pair into one 2-bank tile
                        ps_s = ps_big.tile([128, 1024], F32, tag="big", name="ps_s")
                        for parity in (0, 1):
                            pofs = 64 * parity
                            nc.tensor.matmul(
                                ps_s[:, 512 * parity : 512 * (parity + 1)],
                                lhsT=kT_sb[
                                    pofs : pofs + 64, mt, 128 * kt : 128 * (kt + 1)
                                ],
                                rhs=qT_sb[
                                    pofs : pofs + 64, mt, 512 * qc : 512 * (qc + 1)
                                ],
                                start=True,
                                stop=True,
                            )
                        ex = exp_pool.tile([128, 1024], BF16)
                        nc.scalar.activation(
                            ex,
                            ps_s,
                            mybir.ActivationFunctionType.Exp,
                            scale=1.0 / np.sqrt(HD),
                        )
                        di = kt - 4 * qc
                        if di >= 0:
                            nc.vector.tensor_mul(ex, ex, masks_sb[:, di, :])
                        if prev is not None:
                            _pv(*prev)
                        prev = (ex, kt)
                    _pv(*prev)
                    for parity in (0, 1):
                        # reciprocal_approx_fast (custom DVE ucode) only works
                        # on APs based at partition 0 on this runtime: the odd
                        # head (denom at partition 0) takes 1/r before the
                        # DRAM-bounce broadcast; the even head (denom at
                        # partition 64) broadcasts raw r to partitions 0-63
                        # first and takes the reciprocal there.
                        pofs = 64 * parity
                        r_part = 64 if parity == 0 else 0
                        data_lo = 0 if parity == 0 else 64
                        u = ps_u[parity]
                        ridx = (mt * QC + qc) * 2 + parity
                        rinv = small_pool.tile([128, 512], F32, tag="rinv")
                        if parity == 1:
                            nc.vector.reciprocal_approx_fast(
                                out=rinv[0:1, :], in_=u[0:1, :]
                            )
                            src = rinv[0:1, :]
                        else:
                            nc.vector.tensor_copy(rinv[64:65, :], u[64:65, :])
                            src = rinv[64:65, :]
                        nc.sync.dma_start(out=rscr[ridx : ridx + 1, :], in_=src)
                        rb = small_pool.tile([128, 512], F32, tag="rb")
                        bsrc = bass.AP(
                            tensor=rscr.tensor,
                            offset=rscr[ridx : ridx + 1, :].offset,
                            ap=[[0, 64]] + list(rscr[ridx : ridx + 1, :].ap[1:]),
                        )
                        nc.gpsimd.dma_start(out=rb[pofs : pofs + 64, :], in_=bsrc)
                        if parity == 0:
                            nc.vector.reciprocal_approx_fast(
                                out=rb[0:64, :], in_=rb[0:64, :]
                            )
                        nc.vector.tensor_mul(
                            ctxT_sb[pofs : pofs + 64, mt, 512 * qc : 512 * (qc + 1)],
                            u[data_lo : data_lo + 64, :],
                            rb[pofs : pofs + 64, :],
                        )

                # output projection runs one q-chunk behind the attention so
                # the in-order PE queue never stalls on the normalize chain
                # (U -> reciprocal -> DRAM-bounce broadcast -> ctx multiply).
                for st in _y_tiles_for_iteration(qc):
                    ps = ps_big.tile([128, 1024], F32, tag="big", name="ps")
                    for half in range(2):
                        for kt2 in range(2):
                            nc.tensor.matmul(
                                ps[:, 512 * half : 512 * (half + 1)],
                                lhsT=ctxT_sb[:, kt2, 128 * st : 128 * (st + 1)],
                                rhs=wo_sb[:, kt2, 512 * half : 512 * (half + 1)],
                                start=(kt2 == 0),
                                stop=(kt2 == 1),
                            )
                    ysb = io_pool.tile([128, 1024], F32)
                    nc.vector.tensor_copy(ysb, ps)
                    nc.sync.dma_start(out=y_r[st], in_=ysb)

            if debug:
                for nm, sb in (
                    ("d_qT", qT_sb),
                    ("d_kT", kT_sb),
                    ("d_v", v_sb),
                    ("d_ctxT", ctxT_sb),
                ):
                    flat = sb.rearrange("p a b -> p (a b)")
                    w = flat.shape[1]
                    for off in range(0, w, 512):
                        wid = min(512, w - off)
                        tmp2 = io_pool.tile([128, 1024], F32, tag="dtmp", name="dtmp")
                        nc.vector.tensor_copy(tmp2[:, 0:wid], flat[:, off : off + wid])
                        nc.sync.dma_start(
                            out=dbg[nm][:, off : off + wid], in_=tmp2[:, 0:wid]
                        )
    nc.finalize()
    return nc


_NC = None


def _get_nc():
    global _NC
    if _NC is None:
        _NC = _build_nc()
    return _NC


def kernel(x, Wq, Wk, Wv, Wo):
    x = np.asarray(x, dtype=np.float32)
    bf = ml_dtypes.bfloat16
    in_maps = []
    for c in range(NCORES):
        b, g = divmod(c, 4)
        sl = slice(g * DH, (g + 1) * DH)
        in_maps.append(
            {
                "xT": np.ascontiguousarray(x[b].T).astype(bf),
                "wq": np.ascontiguousarray(np.asarray(Wq)[:, sl]).astype(bf),
                "wk": np.ascontiguousarray(np.asarray(Wk)[:, sl]).astype(bf),
                "wv": np.ascontiguousarray(np.asarray(Wv)[:, sl]).astype(bf),
                "wo": np.ascontiguousarray(np.asarray(Wo)[sl, :]).astype(bf),
            }
        )
    global _last_in_maps
    _last_in_maps = in_maps
    res = run_bass_kernel_spmd(
        _get_nc(), in_maps, core_ids=list(range(NCORES)), trace=False
    )
    out = np.zeros((B, N, D), dtype=np.float32)
    for c in range(NCORES):
        out[c // 4] += res.results[c]["y"]
    return out
